# revision 1
# baseline (speedup 1.0000x reference)
"""nn_DTW kernel for 8 Trainium2 NeuronCores (batch data-parallel).

See _build_cfg for the device program; the host does the sequential
backtrack pointer-chase and the final logsumexp combine.
"""

from contextlib import ExitStack

import concourse.bass as bass
import concourse.bacc as bacc
import concourse.tile as tile
from concourse import mybir
from concourse.masks import make_identity

F32 = mybir.dt.float32
AX = mybir.AxisListType
OP = mybir.AluOpType
ACT = mybir.ActivationFunctionType

BIG = 1.0e30


def _build_cfg(B=8, N=512, M=512, D=256, S=16, W=32, R=8, PART=128):
    assert S * W == M and N % R == 0
    P = S * B
    assert P <= PART
    NT = (N + PART - 1) // PART
    MT = (M + PART - 1) // PART
    DB = (D + PART - 1) // PART
    PN = min(PART, N)
    PD = min(PART, D)
    NSTEP = N // R
    T_TOT = NSTEP + S - 1
    SLOTS = N + R * S
    SLOT = W + 1

    nc = bacc.Bacc("TRN2", target_bir_lowering=False, debug=False)

    x_in = nc.dram_tensor("x", [B, N, D], F32, kind="ExternalInput").ap()
    y_in = nc.dram_tensor("y", [B, M, D], F32, kind="ExternalInput").ap()
    tc_out = nc.dram_tensor("tc_out", [P, SLOTS, SLOT], F32, kind="ExternalOutput").ap()
    neg_out = nc.dram_tensor("neg_out", [B, 1], F32, kind="ExternalOutput").ap()
    cost_stage = nc.dram_tensor("cost_stage", [NT, B, PN, M], F32).ap()

    with tile.TileContext(nc) as tcx, ExitStack() as ctx:
        const = ctx.enter_context(tcx.tile_pool(name="const", bufs=1))
        ident = const.tile([PART, PART], F32)
        make_identity(nc, ident[:])
        oneh = const.tile([PN, B, B], F32)
        nc.vector.memset(oneh[:], 0.0)
        for b_ in range(B):
            nc.vector.memset(oneh[:, b_, b_:b_ + 1], 1.0)
        big_m0 = const.tile([P, W], F32)
        nc.vector.memset(big_m0[:], BIG)
        shift8 = const.tile([PART, PART], F32)
        nc.gpsimd.memset(shift8[:], 0.0)
        nc.gpsimd.affine_select(
            out=shift8[:], in_=shift8[:], compare_op=OP.not_equal, fill=1.0,
            base=B, pattern=[[-1, PART]], channel_multiplier=1,
        )
        bigrow = const.tile([1, PART], F32)
        nc.vector.memset(bigrow[:], 0.0)
        nc.vector.memset(bigrow[0:1, 0:B], BIG)
        onesR = const.tile([1, R], F32)
        nc.vector.memset(onesR[:], 1.0)

        strip = ctx.enter_context(tcx.tile_pool(name="strip", bufs=1))
        tc_strip = strip.tile([P, SLOTS, SLOT], F32)
        nc.gpsimd.memset(tc_strip[:, :, :], BIG)

        # persistent transposed operands + per-batch scales
        oper = ctx.enter_context(tcx.tile_pool(name="oper", bufs=1))
        xTall = oper.tile([PD, B, DB, N], F32)
        ynTall = oper.tile([PD, B, DB, M], F32)
        xrn_all = oper.tile([PN, B, NT], F32)

        # stage-B pools created up-front so B0 can interleave with stage A
        stage = ctx.enter_context(tcx.tile_pool(name="stage", bufs=3))
        neg_pool = ctx.enter_context(tcx.tile_pool(name="negp", bufs=1))
        ps_c = ctx.enter_context(tcx.tile_pool(name="ps_c", bufs=3, space="PSUM"))
        ps_neg = ctx.enter_context(tcx.tile_pool(name="ps_neg", bufs=1, space="PSUM"))
        ngb = ps_neg.tile([B, M], F32, tag="ngb", bufs=1)

        def emit_stageB_batch(nt, b):
            rows = min(PART, N - nt * PART)
            psc = ps_c.tile([PN, M], F32, tag="psc", name=f"psc_{nt}_{b}")
            for db in range(DB):
                dcols = min(PART, D - db * PART)
                nc.tensor.matmul(
                    psc[:rows, :],
                    xTall[:dcols, b, db, nt * PART:nt * PART + rows],
                    ynTall[:dcols, b, db, :],
                    start=(db == 0), stop=(db == DB - 1),
                )
            cn = stage.tile([PN, M], F32, tag="cn", name=f"cn_{nt}_{b}")
            nc.scalar.activation(cn[:rows], psc[:rows], ACT.Copy,
                                 scale=xrn_all[:rows, b, nt:nt + 1], bias=1.0)
            nc.tensor.matmul(
                ngb[:, :],
                oneh[:rows, b, :],
                cn[:rows, :],
                start=(nt == 0 and b == 0),
                stop=(nt == NT - 1 and b == B - 1),
                skip_group_check=True,
            )
            heng = nc.scalar if b % 2 == 0 else nc.sync
            heng.dma_start(out=cost_stage[nt, b], in_=cn[:rows, :])

        def emit_hop2_part(nt, quarter):
            rows = min(PART, N - nt * PART)
            for s in range(quarter * S // 4, (quarter + 1) * S // 4):
                src = cost_stage[nt, :, :, s * W:(s + 1) * W]
                eng = nc.sync if s % 2 == 0 else nc.scalar
                eng.dma_start(
                    out=tc_strip[s * B:s * B + B,
                                 R * s + nt * PART:R * s + nt * PART + rows,
                                 1:SLOT],
                    in_=src)

        def emit_hop2(nt):
            rows = min(PART, N - nt * PART)
            for s in range(S):
                src = cost_stage[nt, :, :, s * W:(s + 1) * W]
                eng = nc.sync if s % 2 == 0 else nc.scalar
                eng.dma_start(
                    out=tc_strip[s * B:s * B + B,
                                 R * s + nt * PART:R * s + nt * PART + rows,
                                 1:SLOT],
                    in_=src)

        def emit_stageB(nt):
            for b in range(B):
                emit_stageB_batch(nt, b)
            emit_hop2(nt)

        # ---------------- Stage A: loads, norms, transposes ----------------
        with ExitStack() as ctxA:
            xy = ctxA.enter_context(tcx.tile_pool(name="xy", bufs=2))
            nrm = ctxA.enter_context(tcx.tile_pool(name="nrm", bufs=3))
            ps_t = ctxA.enter_context(tcx.tile_pool(name="ps_t", bufs=2, space="PSUM"))

            for b in range(B):
                y_all = xy.tile([PN, MT, D], F32, tag="ldy")
                nc.sync.dma_start(
                    out=y_all[:, :, :],
                    in_=y_in[b].rearrange("(t n) d -> n t d", t=MT))
                x_all = xy.tile([PN, NT, D], F32, tag="ldx")
                nc.sync.dma_start(
                    out=x_all[:, :, :],
                    in_=x_in[b].rearrange("(t n) d -> n t d", t=NT))

                ps_y = [ps_t.tile([PD, M], F32, tag=f"pstr{db}", name=f"psy{db}_{b}")
                        for db in range(DB)]
                for mt in range(MT):
                    rows = min(PART, M - mt * PART)
                    yt = y_all[:rows, mt, :]
                    sq = xy.tile([PART, D], F32, tag="sq")
                    s2 = nrm.tile([PART, 1], F32, tag="s2")
                    nc.scalar.activation(sq[:rows], yt, ACT.Square, accum_out=s2[:rows])
                    nrm_t = nrm.tile([PART, 1], F32, tag="nrm")
                    nc.scalar.activation(nrm_t[:rows], s2[:rows], ACT.Sqrt)
                    rn = nrm.tile([PART, 1], F32, tag="rn")
                    nc.vector.reciprocal(rn[:rows], nrm_t[:rows])
                    yn = xy.tile([PART, D], F32, tag="yn")
                    nc.vector.tensor_scalar_mul(yn[:rows], yt, rn[:rows])
                    for db in range(DB):
                        dcols = min(PART, D - db * PART)
                        nc.tensor.transpose(
                            ps_y[db][:dcols, mt * PART:mt * PART + rows],
                            yn[:rows, db * PART:db * PART + dcols],
                            ident[:rows, :rows])
                for db in range(DB):
                    nc.scalar.copy(ynTall[:, b, db, :], ps_y[db][:, :])

                ps_x = [ps_t.tile([PD, N], F32, tag=f"pstr{db}", name=f"psx{db}_{b}")
                        for db in range(DB)]
                for nt in range(NT):
                    rows = min(PART, N - nt * PART)
                    xt = x_all[:rows, nt, :]
                    sq = xy.tile([PART, D], F32, tag="sq")
                    s2 = nrm.tile([PART, 1], F32, tag="s2")
                    nc.scalar.activation(sq[:rows], xt, ACT.Square, accum_out=s2[:rows])
                    nrm_t = nrm.tile([PART, 1], F32, tag="nrm")
                    nc.scalar.activation(nrm_t[:rows], s2[:rows], ACT.Sqrt)
                    rn = nrm.tile([PART, 1], F32, tag="rn")
                    nc.vector.reciprocal(rn[:rows], nrm_t[:rows])
                    nc.vector.tensor_scalar_mul(xrn_all[:rows, b, nt:nt + 1],
                                                rn[:rows], -1.0)
                    for db in range(DB):
                        dcols = min(PART, D - db * PART)
                        nc.tensor.transpose(
                            ps_x[db][:dcols, nt * PART:nt * PART + rows],
                            xt[:, db * PART:db * PART + dcols],
                            ident[:rows, :rows])
                for db in range(DB):
                    nc.scalar.copy(xTall[:, b, db, :], ps_x[db][:, :])
                # interleave the first row-block's cost work for this batch
                emit_stageB_batch(0, b)
            emit_hop2(0)

        # ---------------- Stage C: skew-R DTW wavefront ----------------
        ps_carry = ctx.enter_context(tcx.tile_pool(name="ps_cr", bufs=1, space="PSUM"))
        mpool = ctx.enter_context(tcx.tile_pool(name="mpool", bufs=8))

        NCARRY = 4
        carry_tiles = [
            ps_carry.tile([P, R], F32, tag=f"cr{i}", name=f"carry{i}")
            for i in range(NCARRY)
        ]

        def emit_carry(U, c0, c1):
            base = R * U
            bnd = min(S - 1, U + 1) * B
            cps = carry_tiles[(U + 1) % NCARRY]
            nc.tensor.matmul(
                cps[0:bnd + B, c0:c1],
                shift8[0:bnd, 0:bnd + B],
                tc_strip[0:bnd, base + c0:base + c1, SLOT - 1:SLOT],
                start=True, stop=False, skip_group_check=True,
            )
            nc.tensor.matmul(
                cps[0:bnd + B, c0:c1],
                bigrow[0:1, 0:bnd + B],
                onesR[0:1, 0:c1 - c0],
                start=False, stop=True, skip_group_check=True,
            )
            return cps

        state = {"prev_carry": None, "out_lo": 0}

        def emit_stageC(U0, U1, sprinkle=None):
            for U in range(U0, U1):
                if sprinkle:
                    for off, fn in sprinkle:
                        if U == U0 + off:
                            fn()
                smax = min(S - 1, U)
                phi = (smax + 1) * B
                base = R * U

                for k in range(R):
                    q = base + k
                    if U == 0 and k == 0:
                        m_ap = big_m0[0:phi, :]
                    else:
                        mt_ = mpool.tile([P, W], F32, tag="m", name=f"m_{U}_{k}")
                        nc.vector.tensor_tensor(
                            mt_[0:phi, :],
                            tc_strip[0:phi, q - 1, 0:W],
                            tc_strip[0:phi, q - 1, 1:SLOT],
                            OP.min,
                        )
                        m_ap = mt_[0:phi, :]

                    if U == 0:
                        init = 0.0 if k == 0 else BIG
                    else:
                        init = state["prev_carry"][0:phi, k:k + 1]
                    nc.vector.tensor_tensor_scan(
                        tc_strip[0:phi, q, 1:SLOT],
                        m_ap,
                        tc_strip[0:phi, q, 1:SLOT],
                        init,
                        OP.min,
                        OP.add,
                    )
                    if U + 1 < T_TOT:
                        if k == R - 2:
                            state["cps"] = emit_carry(U, 0, R - 1)
                        elif k == R - 1:
                            cps = emit_carry(U, R - 1, R)
                            bnd = min(S - 1, U + 1) * B
                            nc.scalar.copy(
                                tc_strip[0:bnd + B, base + R:base + 2 * R, 0:1],
                                cps[0:bnd + B, 0:R])
                            state["prev_carry"] = state["cps"]
                # stream finished slots out every 16 supersteps
                if (U + 1) % 8 == 0 and U + 1 < T_TOT:
                    lo, hi = state["out_lo"], (U + 1) * R
                    nc.sync.dma_start(out=tc_out[:, lo:hi, :],
                                      in_=tc_strip[:, lo:hi, :])
                    state["out_lo"] = hi

        # Interleave stage-B blocks with stage-C chunks so each engine's
        # in-order queue pipelines across stages. C-chunk for block nt covers
        # supersteps [nt*PART/R, (nt+1)*PART/R).
        UPB = PART // R                 # supersteps per row-block
        for nt in range(1, NT):
            # spread block nt's batches across chunk nt-1's supersteps
            spr = [(min(2 * b_, UPB - 4), (lambda n_, bb: lambda: emit_stageB_batch(n_, bb))(nt, b_))
                   for b_ in range(B)]
            for qi in range(4):
                spr.append((UPB - 3 + min(qi, 2),
                            (lambda n_, q_: lambda: emit_hop2_part(n_, q_))(nt, qi)))
            emit_stageC((nt - 1) * UPB, nt * UPB, sprinkle=spr)
        emit_stageC((NT - 1) * UPB, T_TOT)

        # neg = logsumexp over m (emitted last; only needed at the end)
        negsum = neg_pool.tile([B, M], F32)
        nc.scalar.copy(negsum[:, :], ngb[:, :])
        mx = neg_pool.tile([B, 1], F32)
        nc.vector.reduce_max(mx[:], negsum[:], AX.X)
        sh = neg_pool.tile([B, M], F32)
        nc.vector.tensor_scalar(sh[:], negsum[:], mx[:], None, OP.subtract)
        ex = neg_pool.tile([B, M], F32)
        esum = neg_pool.tile([B, 1], F32)
        nc.scalar.activation(ex[:], sh[:], ACT.Exp, accum_out=esum[:])
        lg = neg_pool.tile([B, 1], F32)
        nc.scalar.activation(lg[:], esum[:], ACT.Ln)
        negv = neg_pool.tile([B, 1], F32)
        nc.vector.tensor_add(negv[:], lg[:], mx[:])
        nc.sync.dma_start(out=neg_out[:, :], in_=negv[:])

        lo = state["out_lo"]
        nc.sync.dma_start(out=tc_out[:, lo:SLOTS, :], in_=tc_strip[:, lo:SLOTS, :])

    nc.compile()
    return nc


# ---------------------------------------------------------------------------
# Host-side driver: sharding, run, unskew, backtrack walk, final loss
# ---------------------------------------------------------------------------
import numpy as np

B_TOT, N_G, M_G, D_G = 64, 512, 512, 256
N_CORES = 8
B_LOC = B_TOT // N_CORES
S_G, W_G, R_G = 16, 32, 8
P_G = S_G * B_LOC
SLOTS_G = N_G + R_G * S_G
SLOT_G = W_G + 1

_NC_CACHE = {}


def _get_nc():
    if "nc" not in _NC_CACHE:
        _NC_CACHE["nc"] = _build_cfg(B=B_LOC, N=N_G, M=M_G, D=D_G,
                                     S=S_G, W=W_G, R=R_G)
    return _NC_CACHE["nc"]


def _unskew(tc_skew):
    tc = np.empty((B_LOC, N_G, M_G), np.float32)
    for s in range(S_G):
        for b in range(B_LOC):
            tc[b, :, s * W_G:(s + 1) * W_G] = \
                tc_skew[s * B_LOC + b, R_G * s:R_G * s + N_G, 1:SLOT_G]
    return tc


def _host_finish(tc, x, y, neg):
    """Backtrack walk on the device tc + pos logsumexp (host side)."""
    Bt, Nn, Mm = tc.shape
    eps = 1e-8
    xn = x / np.maximum(np.linalg.norm(x, axis=-1, keepdims=True), eps)
    yn = y / np.maximum(np.linalg.norm(y, axis=-1, keepdims=True), eps)
    bidx = np.arange(Bt)
    i = np.full(Bt, Nn - 1, np.int64)
    j = np.full(Bt, Mm - 1, np.int64)
    Is, Js, Vs = [i.copy()], [j.copy()], [np.ones(Bt, bool)]
    active = (i > 0) & (j > 0)
    while active.any():
        a = tc[bidx, np.maximum(i - 1, 0), np.maximum(j - 1, 0)]
        bb = tc[bidx, np.maximum(i - 1, 0), j]
        c = tc[bidx, i, np.maximum(j - 1, 0)]
        diag = (a <= bb) & (a <= c)
        up = (~diag) & (bb <= c)
        ni = np.where(diag | up, i - 1, i)
        nj = np.where(diag | (~up), j - 1, j)
        i = np.where(active, ni, i)
        j = np.where(active, nj, j)
        Is.append(i.copy())
        Js.append(j.copy())
        Vs.append(active.copy())
        active = (i > 0) & (j > 0)
    at00 = (i == 0) & (j == 0)
    Is.append(np.zeros(Bt, np.int64))
    Js.append(np.zeros(Bt, np.int64))
    Vs.append(~at00)

    IS = np.stack(Is, 1)
    JS = np.stack(Js, 1)
    VS = np.stack(Vs, 1)
    costs = 1.0 - np.einsum("bld,bld->bl",
                            xn[bidx[:, None], IS], yn[bidx[:, None], JS])
    colsum = np.zeros((Bt, Mm), np.float32)
    np.add.at(colsum, (bidx[:, None], JS),
              np.where(VS, costs, 0.0).astype(np.float32))
    mxv = colsum.max(axis=1, keepdims=True)
    pos = (mxv + np.log(np.sum(np.exp(colsum - mxv),
                               axis=1, keepdims=True))).squeeze(1)
    return (pos.astype(np.float32) - neg).astype(np.float32)


def run_device(x, y, **kw):
    from concourse import bass_utils

    nc = _get_nc()
    in_maps = [
        {"x": np.ascontiguousarray(x[c * B_LOC:(c + 1) * B_LOC]),
         "y": np.ascontiguousarray(y[c * B_LOC:(c + 1) * B_LOC])}
        for c in range(N_CORES)
    ]
    res = bass_utils.run_bass_kernel_spmd(nc, in_maps, list(range(N_CORES)), **kw)
    tc = np.empty((B_TOT, N_G, M_G), np.float32)
    neg = np.empty(B_TOT, np.float32)
    for c in range(N_CORES):
        out = res.results[c]
        tc[c * B_LOC:(c + 1) * B_LOC] = _unskew(out["tc_out"])
        neg[c * B_LOC:(c + 1) * B_LOC] = out["neg_out"].reshape(B_LOC)
    return tc, neg, res


def kernel(x, y):
    x = np.asarray(x, dtype=np.float32)
    y = np.asarray(y, dtype=np.float32)
    tc, neg, _ = run_device(x, y)
    return _host_finish(tc, x, y, neg)

